# revision 75
# baseline (speedup 1.0000x reference)
"""Causal self-attention (GQA + RoPE) Trainium2 Bass kernel.

Problem: B=2, T=2048, D=2048, H=16 q-heads, KVH=4 kv-heads, HD=128.
Sharding: 8 cores = batch(2) x kv-groups(4). Core (b, g) computes q-heads
4g..4g+3 and kv-head g for batch b, producing a partial output
y_g @ Wo[512g:512g+512, :]; the host sums the 4 partials per batch.

Device-side layout (per core):
  xT   = x[b].T                       [D, T]     (host-transposed)
  qkT  = wqk.T @ xT                   5x[128, T] (q/k transposed: head-dim on partitions)
  v    = x[b] @ Wv_g                  [T, 128]   (natural)
  RoPE via pair-swap matmul (PERM) + elementwise maps C/S (host-precomputed)
  S^T tiles = kT_tile.T @ qT          [tk=128, tq<=512]  -> exp (ACT, scaled) -> P^T
  y^T[:, sl] += v_tile.T @ P^T ;  l[1, sl] += ones.T @ P^T   (PSUM accumulate)
  y^T *= broadcast(1/l)  (GPSIMD partition_broadcast of DVE reciprocal + DVE mul)
  outp = sum_c yT_c.T @ Wo_g[c-block]            [T, D]  partial

Slice-pipelined: for each 512-query slice, 4 heads' attention -> this
slice's output-projection rows -> next slice's rope, so PE/ACT/DVE/DMA
overlap across stages (rope DVE work drains while PE projects; v is
projected first so its PE transposes unblock early; output rows DMA out
per 512-column chunk as each PSUM copy lands). The first two projection
slices run kb-outer so matmuls consume each weight/x tile as its DMA
lands. PSUM budget
(8 banks): qk-proj 6 + v-transpose 2 during projection; then stp 4 +
yps/ops shared 2 + swp 1 + lps 1 (yps and ops have disjoint windows
within a slice, so sharing their two banks double-buffers both).

All matmuls run as float32r (FP22 single-pass); matmul-feeding tiles are
declared float32r and every producer writes the float32r view (walrus
birverifier requires rounded producers). Causality is handled by narrowing
the diagonal tiles' matmuls to the valid column range plus a triangular
mask add on the 128-wide diagonal block.
"""

import sys
import numpy as np

if "/opt/trn_rl_repo" not in sys.path:
    sys.path.insert(0, "/opt/trn_rl_repo")

B, T, D = 2, 2048, 2048
H, KVH = 16, 4
HD = 128
P = 128
NKB = D // P            # 16 contraction blocks
NTK = T // P            # 16 key tiles
NSL = T // 512          # 4 query slices of 512
SCALE = float(1.0 / np.sqrt(HD))
NEG = -1.0e30

_CACHE = {}


def _build_nc():
    import concourse.mybir as mybir
    import concourse.tile as tile
    from concourse import bacc
    from contextlib import ExitStack

    F32 = mybir.dt.float32
    FR = mybir.dt.float32r
    Exp = mybir.ActivationFunctionType.Exp

    nc = bacc.Bacc("TRN2", target_bir_lowering=False, debug=False)

    xT_d = nc.dram_tensor("xT", [D, T], F32, kind="ExternalInput").ap()
    wqkv_d = nc.dram_tensor("wqkv", [D, 768], F32, kind="ExternalInput").ap()
    wo_d = nc.dram_tensor("wo", [512, D], F32, kind="ExternalInput").ap()
    ropeC_d = nc.dram_tensor("ropeC", [P, T], F32, kind="ExternalInput").ap()
    ropeS_d = nc.dram_tensor("ropeS", [P, T], F32, kind="ExternalInput").ap()
    perm_d = nc.dram_tensor("perm", [P, P], F32, kind="ExternalInput").ap()
    tri_d = nc.dram_tensor("tri", [P, P], F32, kind="ExternalInput").ap()
    onesc_d = nc.dram_tensor("onesc", [P, 1], F32, kind="ExternalInput").ap()
    onesr_d = nc.dram_tensor("onesr", [1, P], F32, kind="ExternalInput").ap()
    ident_d = nc.dram_tensor("ident", [P, P], F32, kind="ExternalInput").ap()
    outp_d = nc.dram_tensor("outp", [T, D], F32, kind="ExternalOutput").ap()

    with tile.TileContext(nc) as tc, ExitStack() as ctx:
        # ---- persistent pools -------------------------------------------
        singles = ctx.enter_context(tc.tile_pool(name="singles", bufs=1))
        qk_pool = ctx.enter_context(tc.tile_pool(name="qk", bufs=1))
        v_pool = ctx.enter_context(tc.tile_pool(name="vp", bufs=1))

        ropeC = singles.tile([P, T], F32)
        ropeS = singles.tile([P, T], F32)
        perm = singles.tile([P, P], FR)
        nc.gpsimd.dma_start(out=perm, in_=perm_d.bitcast(FR))
        tri = singles.tile([P, P], F32)
        nc.gpsimd.dma_start(out=tri, in_=tri_d)
        onesc = singles.tile([P, 1], FR)
        nc.gpsimd.dma_start(out=onesc, in_=onesc_d.bitcast(FR))
        onesr = singles.tile([1, P], FR)
        nc.gpsimd.dma_start(out=onesr, in_=onesr_d.bitcast(FR))
        ident = singles.tile([P, P], F32)
        nc.gpsimd.dma_start(out=ident, in_=ident_d)

        # qkT[m]: m=0..3 q-heads, m=4 k-head; vT = v transposed [HD, T];
        # v_sb[:, tk, :] = v[128tk:128tk+128, :] natural (for PV lhsT)
        qkT = [qk_pool.tile([P, T], FR, name=f"qkT{m}") for m in range(5)]
        vT = qk_pool.tile([P, T], F32, name="vT")
        v_sb = v_pool.tile([P, NTK, HD], FR)

        # ---- phase 1: projections + v transpose --------------------------
        with tc.tile_pool(name="wqkv", bufs=1) as wpool, \
             tc.tile_pool(name="xts", bufs=32) as xpool, \
             tc.tile_pool(name="qkps", bufs=1, space="PSUM") as qkps_pool, \
             tc.tile_pool(name="vtps", bufs=2, space="PSUM") as vtpool:
            wqkv_sb = wpool.tile([P, NKB, 768], FR)
            wqkv_r = wqkv_d.rearrange("(kb p) m -> p kb m", p=P).bitcast(FR)
            for kb in range(NKB):
                nc.scalar.dma_start(out=wqkv_sb[:, kb, :], in_=wqkv_r[:, kb, :])

            for n in range(NSL):
                xts = []
                for kb in range(NKB):
                    xt = xpool.tile([P, 512], FR, name="xt", tag="xt")
                    nc.sync.dma_start(
                        out=xt,
                        in_=xT_d[kb * P:(kb + 1) * P, n * 512:(n + 1) * 512].bitcast(FR))
                    xts.append(xt)
                if n <= 1:
                    # kb-outer on the first two slices: consume each weight and
                    # xt tile as its DMA lands instead of waiting for the batch
                    qkps6 = [qkps_pool.tile([P, 512], F32, name=f"qkps{m}",
                                            tag=f"qkps{m}") for m in range(6)]
                    for kb in range(NKB):
                        for m in range(6):
                            nc.tensor.matmul(
                                qkps6[m], lhsT=wqkv_sb[:, kb, m * P:(m + 1) * P],
                                rhs=xts[kb], start=(kb == 0), stop=(kb == NKB - 1))
                    nc.any.tensor_copy(
                        out=vT[:, n * 512:(n + 1) * 512], in_=qkps6[5])
                    for m in range(5):
                        nc.any.tensor_copy(
                            out=qkT[m][:, n * 512:(n + 1) * 512], in_=qkps6[m])
                else:
                    # m-outer: each m's psum group closes early so its copy-out
                    # overlaps the next m's matmuls
                    for m in (5, 0, 1, 2, 3, 4):
                        qkps = qkps_pool.tile([P, 512], F32, name=f"qkps{m}",
                                              tag=f"qkps{m}")
                        for kb in range(NKB):
                            nc.tensor.matmul(
                                qkps, lhsT=wqkv_sb[:, kb, m * P:(m + 1) * P],
                                rhs=xts[kb], start=(kb == 0), stop=(kb == NKB - 1))
                        if m < 5:
                            nc.any.tensor_copy(
                                out=qkT[m][:, n * 512:(n + 1) * 512], in_=qkps)
                        else:
                            nc.any.tensor_copy(
                                out=vT[:, n * 512:(n + 1) * 512], in_=qkps)
                for tk in range(4 * n, 4 * n + 4):
                    vtp = vtpool.tile([P, P], F32, name="vtp", tag="vtp")
                    nc.tensor.transpose(vtp, in_=vT[:, tk * P:(tk + 1) * P],
                                        identity=ident)
                    nc.any.tensor_copy(out=v_sb[:, tk, :], in_=vtp)
                if n == 0:
                    # rope maps: needed from the rope of slice 0 (~25us in);
                    # emitted after slice-0 proj so they don't delay startup
                    nc.scalar.dma_start(out=ropeC, in_=ropeC_d)
                    nc.scalar.dma_start(out=ropeS, in_=ropeS_d)

        # wo load here: overlaps rope + early attention, after proj DMAs
        y_pool = ctx.enter_context(tc.tile_pool(name="yp", bufs=1))
        yT = [y_pool.tile([P, T], FR, name=f"yT{h}") for h in range(4)]
        wos_pool = ctx.enter_context(tc.tile_pool(name="wos", bufs=1))
        wo_sb = wos_pool.tile([P, 4, D], FR)
        nc.scalar.dma_start(
            out=wo_sb, in_=wo_d.rearrange("(c p) d -> p c d", p=P).bitcast(FR))

        # ---- phase 2+3: rope, attention, output proj, slice-pipelined ----
        with tc.tile_pool(name="ropet", bufs=3) as rpool, \
             tc.tile_pool(name="pts", bufs=4) as ptpool, \
             tc.tile_pool(name="lsb", bufs=2) as lpool, \
             tc.tile_pool(name="ytmp", bufs=3) as ytpool, \
             tc.tile_pool(name="osb", bufs=3) as opool, \
             tc.tile_pool(name="ps", bufs=1, space="PSUM") as ps:
            kT = qkT[4]

            def rope_slice(n):
                sl = slice(n * 512, (n + 1) * 512)
                for m in (4, 0, 1, 2, 3):
                    swp = ps.tile([P, 512], F32, name="swp", tag="swp", bufs=1)
                    nc.tensor.matmul(swp, lhsT=perm, rhs=qkT[m][:, sl],
                                     start=True, stop=True)
                    t1 = rpool.tile([P, 512], F32, name="t1", tag="t1")
                    nc.vector.tensor_mul(t1, qkT[m][:, sl].bitcast(F32), ropeC[:, sl])
                    t2 = rpool.tile([P, 512], F32, name="t2", tag="t2")
                    nc.vector.tensor_mul(t2, swp, ropeS[:, sl])
                    nc.vector.tensor_add(qkT[m][:, sl], t1, t2)

            rope_slice(0)
            for n in range(NSL):
                sl = slice(n * 512, (n + 1) * 512)
                # attention for all 4 heads on this slice
                ntk = 4 * n + 4
                for h in range(4):
                    qTh = qkT[h]
                    yps = ps.tile([P, 512], F32, name="yps", tag="ypo", bufs=2)
                    lps = ps.tile([1, 512], F32, name="lps", tag="lps", bufs=1)
                    for tk in range(ntk):
                        j = tk - 4 * n
                        c0 = j * P if j >= 1 else 0    # first valid local column
                        stp = ps.tile([P, 512], F32, name="stp", tag="stp", bufs=4)
                        nc.tensor.matmul(
                            stp[:, c0:], lhsT=kT[:, tk * P:(tk + 1) * P],
                            rhs=qTh[:, n * 512 + c0:(n + 1) * 512],
                            start=True, stop=True)
                        if j >= 0:
                            nc.vector.tensor_add(
                                stp[:, c0:c0 + P], stp[:, c0:c0 + P], tri)
                        pt = ptpool.tile([P, 512], FR, name="pt", tag="pt")
                        nc.scalar.activation(out=pt[:, c0:], in_=stp[:, c0:],
                                             func=Exp, scale=SCALE)
                        st, sp = (tk == 0), (tk == ntk - 1)
                        nc.tensor.matmul(yps[:, c0:], lhsT=v_sb[:, tk, :],
                                         rhs=pt[:, c0:], start=st, stop=sp)
                        nc.tensor.matmul(lps[:, c0:], lhsT=onesc,
                                         rhs=pt[:, c0:], start=st, stop=sp)
                    linv = lpool.tile([1, 512], F32, name="linv", tag="linv")
                    nc.vector.reciprocal(out=linv, in_=lps)
                    bcsb = ytpool.tile([P, 512], F32, name="bcsb", tag="bcsb")
                    nc.gpsimd.partition_broadcast(bcsb, linv)
                    ysb = ytpool.tile([P, 512], F32, name="ysb", tag="ysb")
                    nc.any.tensor_copy(out=ysb, in_=yps)
                    nc.vector.tensor_mul(yT[h][:, sl], ysb, bcsb)
                # output projection for this slice's 4 row-tiles
                for t in range(4 * n, 4 * n + 4):
                    outsb = opool.tile([P, D], F32, name="outsb", tag="outsb")
                    for dsl in range(4):
                        ops = ps.tile([P, 512], F32, name="ops", tag="ypo", bufs=2)
                        for c in range(4):
                            nc.tensor.matmul(
                                ops, lhsT=yT[c][:, t * P:(t + 1) * P],
                                rhs=wo_sb[:, c, dsl * 512:(dsl + 1) * 512],
                                start=(c == 0), stop=(c == 3))
                        nc.any.tensor_copy(
                            out=outsb[:, dsl * 512:(dsl + 1) * 512], in_=ops)
                        nc.sync.dma_start(
                            out=outp_d[t * P:(t + 1) * P, dsl * 512:(dsl + 1) * 512],
                            in_=outsb[:, dsl * 512:(dsl + 1) * 512])
                # next slice's rope after this slice's output projection
                if t == 4 * n + 3 and dsl == 3 and n + 1 < NSL:
                    rope_slice(n + 1)

    nc.compile()
    return nc


def _host_consts(freqs_cos, freqs_sin):
    C = np.repeat(np.asarray(freqs_cos, np.float32).T, 2, axis=0)
    S = np.repeat(np.asarray(freqs_sin, np.float32).T, 2, axis=0).copy()
    S[0::2] *= -1.0
    C = np.ascontiguousarray(C)
    S = np.ascontiguousarray(S)
    perm = np.zeros((P, P), np.float32)
    perm[np.arange(P), np.arange(P) ^ 1] = 1.0
    tri = np.where(np.arange(P)[:, None] <= np.arange(P)[None, :], 0.0, NEG).astype(np.float32)
    onesc = np.ones((P, 1), np.float32)
    onesr = np.ones((1, P), np.float32)
    return C, S, perm, tri, onesc, onesr


def _in_maps(x, freqs_cos, freqs_sin, Wq, Wk, Wv, Wo):
    C, S, perm, tri, onesc, onesr = _host_consts(freqs_cos, freqs_sin)
    xTb = [np.ascontiguousarray(np.asarray(x, np.float32)[b].T) for b in range(B)]
    Wq = np.asarray(Wq, np.float32); Wk = np.asarray(Wk, np.float32)
    Wv = np.asarray(Wv, np.float32); Wo = np.asarray(Wo, np.float32)
    ident = np.eye(P, dtype=np.float32)
    maps = []
    for b in range(B):
        for g in range(KVH):
            wqkv = np.ascontiguousarray(np.concatenate(
                [Wq[:, 512 * g:512 * (g + 1)], Wk[:, HD * g:HD * (g + 1)],
                 Wv[:, HD * g:HD * (g + 1)]], axis=1))
            maps.append({
                "xT": xTb[b],
                "wqkv": wqkv,
                "wo": np.ascontiguousarray(Wo[512 * g:512 * (g + 1), :]),
                "ropeC": C, "ropeS": S, "perm": perm, "tri": tri,
                "onesc": onesc, "onesr": onesr, "ident": ident,
            })
    return maps


def kernel(x, freqs_cos, freqs_sin, Wq, Wk, Wv, Wo):
    from concourse.bass_utils import run_bass_kernel_spmd

    if "nc" not in _CACHE:
        _CACHE["nc"] = _build_nc()
    nc = _CACHE["nc"]

    in_maps = _in_maps(x, freqs_cos, freqs_sin, Wq, Wk, Wv, Wo)
    res = None
    for attempt in range(3):
        try:
            res = run_bass_kernel_spmd(nc, in_maps, core_ids=list(range(8)))
            break
        except Exception:
            if attempt == 2:
                raise
    assert res is not None
    out = np.zeros((B, T, D), np.float32)
    for b in range(B):
        for g in range(KVH):
            out[b] += res.results[b * KVH + g]["outp"]
    return out


# revision 79
# speedup vs baseline: 1.0012x; 1.0012x over previous
"""Causal self-attention (GQA + RoPE) Trainium2 Bass kernel.

Problem: B=2, T=2048, D=2048, H=16 q-heads, KVH=4 kv-heads, HD=128.
Sharding: 8 cores = batch(2) x kv-groups(4). Core (b, g) computes q-heads
4g..4g+3 and kv-head g for batch b, producing a partial output
y_g @ Wo[512g:512g+512, :]; the host sums the 4 partials per batch.

Device-side layout (per core):
  xT   = x[b].T                       [D, T]     (host-transposed)
  qkT  = wqk.T @ xT                   5x[128, T] (q/k transposed: head-dim on partitions)
  v    = x[b] @ Wv_g                  [T, 128]   (natural)
  RoPE via pair-swap matmul (PERM) + elementwise maps C/S (host-precomputed)
  S^T tiles = kT_tile.T @ qT          [tk=128, tq<=512]  -> exp (ACT, scaled) -> P^T
  y^T[:, sl] += v_tile.T @ P^T ;  l[1, sl] += ones.T @ P^T   (PSUM accumulate)
  y^T *= broadcast(1/l)  (GPSIMD partition_broadcast of DVE reciprocal + DVE mul)
  outp = sum_c yT_c.T @ Wo_g[c-block]            [T, D]  partial

Slice-pipelined: for each 512-query slice, 4 heads' attention -> this
slice's output-projection rows -> next slice's rope, so PE/ACT/DVE/DMA
overlap across stages (rope DVE work drains while PE projects; v is
projected first so its PE transposes unblock early; output rows DMA out
per 512-column chunk as each PSUM copy lands). The first two projection
slices run kb-outer so matmuls consume each weight/x tile as its DMA
lands. PSUM budget
(8 banks): qk-proj 6 + v-transpose 2 during projection; then stp 4 +
yps/ops shared 2 + swp 1 + lps 1 (yps and ops have disjoint windows
within a slice, so sharing their two banks double-buffers both).

All matmuls run as float32r (FP22 single-pass); matmul-feeding tiles are
declared float32r and every producer writes the float32r view (walrus
birverifier requires rounded producers). Causality is handled by narrowing
the diagonal tiles' matmuls to the valid column range plus a triangular
mask add on the 128-wide diagonal block.
"""

import sys
import numpy as np

if "/opt/trn_rl_repo" not in sys.path:
    sys.path.insert(0, "/opt/trn_rl_repo")

B, T, D = 2, 2048, 2048
H, KVH = 16, 4
HD = 128
P = 128
NKB = D // P            # 16 contraction blocks
NTK = T // P            # 16 key tiles
NSL = T // 512          # 4 query slices of 512
SCALE = float(1.0 / np.sqrt(HD))
NEG = -1.0e30

_CACHE = {}


def _build_nc():
    import concourse.mybir as mybir
    import concourse.tile as tile
    from concourse import bacc
    from contextlib import ExitStack

    F32 = mybir.dt.float32
    FR = mybir.dt.float32r
    Exp = mybir.ActivationFunctionType.Exp

    nc = bacc.Bacc("TRN2", target_bir_lowering=False, debug=False)

    xT_d = nc.dram_tensor("xT", [D, T], F32, kind="ExternalInput").ap()
    wqkv_d = nc.dram_tensor("wqkv", [D, 768], F32, kind="ExternalInput").ap()
    wo_d = nc.dram_tensor("wo", [512, D], F32, kind="ExternalInput").ap()
    ropeC_d = nc.dram_tensor("ropeC", [P, T], F32, kind="ExternalInput").ap()
    ropeS_d = nc.dram_tensor("ropeS", [P, T], F32, kind="ExternalInput").ap()
    perm_d = nc.dram_tensor("perm", [P, P], F32, kind="ExternalInput").ap()
    tri_d = nc.dram_tensor("tri", [P, P], F32, kind="ExternalInput").ap()
    onesc_d = nc.dram_tensor("onesc", [P, 1], F32, kind="ExternalInput").ap()
    onesr_d = nc.dram_tensor("onesr", [1, P], F32, kind="ExternalInput").ap()
    ident_d = nc.dram_tensor("ident", [P, P], F32, kind="ExternalInput").ap()
    outp_d = nc.dram_tensor("outp", [T, D], F32, kind="ExternalOutput").ap()

    with tile.TileContext(nc) as tc, ExitStack() as ctx:
        # ---- persistent pools -------------------------------------------
        singles = ctx.enter_context(tc.tile_pool(name="singles", bufs=1))
        qk_pool = ctx.enter_context(tc.tile_pool(name="qk", bufs=1))
        v_pool = ctx.enter_context(tc.tile_pool(name="vp", bufs=1))

        ropeC = singles.tile([P, T], F32)
        ropeS = singles.tile([P, T], F32)
        perm = singles.tile([P, P], FR)
        nc.gpsimd.dma_start(out=perm, in_=perm_d.bitcast(FR))
        tri = singles.tile([P, P], F32)
        nc.gpsimd.dma_start(out=tri, in_=tri_d)
        onesc = singles.tile([P, 1], FR)
        nc.gpsimd.dma_start(out=onesc, in_=onesc_d.bitcast(FR))
        onesr = singles.tile([1, P], FR)
        nc.gpsimd.dma_start(out=onesr, in_=onesr_d.bitcast(FR))
        ident = singles.tile([P, P], F32)
        nc.gpsimd.dma_start(out=ident, in_=ident_d)

        # qkT[m]: m=0..3 q-heads, m=4 k-head; vT = v transposed [HD, T];
        # v_sb[:, tk, :] = v[128tk:128tk+128, :] natural (for PV lhsT)
        qkT = [qk_pool.tile([P, T], FR, name=f"qkT{m}") for m in range(5)]
        vT = qk_pool.tile([P, T], F32, name="vT")
        v_sb = v_pool.tile([P, NTK, HD], FR)

        # ---- phase 1: projections + v transpose --------------------------
        with tc.tile_pool(name="wqkv", bufs=1) as wpool, \
             tc.tile_pool(name="xts", bufs=32) as xpool, \
             tc.tile_pool(name="qkps", bufs=1, space="PSUM") as qkps_pool, \
             tc.tile_pool(name="vtps", bufs=2, space="PSUM") as vtpool:
            wqkv_sb = wpool.tile([P, NKB, 768], FR)
            wqkv_r = wqkv_d.rearrange("(kb p) m -> p kb m", p=P).bitcast(FR)
            for kb in range(NKB):
                nc.scalar.dma_start(out=wqkv_sb[:, kb, :], in_=wqkv_r[:, kb, :])

            for n in range(NSL):
                xts = []
                for kb in range(NKB):
                    xt = xpool.tile([P, 512], FR, name="xt", tag="xt")
                    nc.sync.dma_start(
                        out=xt,
                        in_=xT_d[kb * P:(kb + 1) * P, n * 512:(n + 1) * 512].bitcast(FR))
                    xts.append(xt)
                if n <= 1:
                    # kb-outer on the first two slices: consume each weight and
                    # xt tile as its DMA lands instead of waiting for the batch
                    qkps6 = [qkps_pool.tile([P, 512], F32, name=f"qkps{m}",
                                            tag=f"qkps{m}") for m in range(6)]
                    for kb in range(NKB):
                        for m in range(6):
                            nc.tensor.matmul(
                                qkps6[m], lhsT=wqkv_sb[:, kb, m * P:(m + 1) * P],
                                rhs=xts[kb], start=(kb == 0), stop=(kb == NKB - 1))
                    nc.any.tensor_copy(
                        out=vT[:, n * 512:(n + 1) * 512], in_=qkps6[5])
                    for m in range(5):
                        nc.any.tensor_copy(
                            out=qkT[m][:, n * 512:(n + 1) * 512], in_=qkps6[m])
                else:
                    # m-outer: each m's psum group closes early so its copy-out
                    # overlaps the next m's matmuls
                    for m in (5, 0, 1, 2, 3, 4):
                        qkps = qkps_pool.tile([P, 512], F32, name=f"qkps{m}",
                                              tag=f"qkps{m}")
                        for kb in range(NKB):
                            nc.tensor.matmul(
                                qkps, lhsT=wqkv_sb[:, kb, m * P:(m + 1) * P],
                                rhs=xts[kb], start=(kb == 0), stop=(kb == NKB - 1))
                        if m < 5:
                            nc.any.tensor_copy(
                                out=qkT[m][:, n * 512:(n + 1) * 512], in_=qkps)
                        else:
                            nc.any.tensor_copy(
                                out=vT[:, n * 512:(n + 1) * 512], in_=qkps)
                for tk in range(4 * n, 4 * n + 4):
                    vtp = vtpool.tile([P, P], F32, name="vtp", tag="vtp")
                    nc.tensor.transpose(vtp, in_=vT[:, tk * P:(tk + 1) * P],
                                        identity=ident)
                    nc.any.tensor_copy(out=v_sb[:, tk, :], in_=vtp)
                if n == 0:
                    # rope maps: needed from the rope of slice 0 (~25us in);
                    # emitted after slice-0 proj so they don't delay startup
                    nc.scalar.dma_start(out=ropeC, in_=ropeC_d)
                    nc.scalar.dma_start(out=ropeS, in_=ropeS_d)

        # wo load here: overlaps rope + early attention, after proj DMAs
        y_pool = ctx.enter_context(tc.tile_pool(name="yp", bufs=1))
        yT = [y_pool.tile([P, T], FR, name=f"yT{h}") for h in range(4)]
        wos_pool = ctx.enter_context(tc.tile_pool(name="wos", bufs=1))
        wo_sb = wos_pool.tile([P, 4, D], FR)
        nc.scalar.dma_start(
            out=wo_sb, in_=wo_d.rearrange("(c p) d -> p c d", p=P).bitcast(FR))

        # ---- phase 2+3: rope, attention, output proj, slice-pipelined ----
        with tc.tile_pool(name="ropet", bufs=3) as rpool, \
             tc.tile_pool(name="pts", bufs=4) as ptpool, \
             tc.tile_pool(name="lsb", bufs=2) as lpool, \
             tc.tile_pool(name="ytmp", bufs=3) as ytpool, \
             tc.tile_pool(name="osb", bufs=3) as opool, \
             tc.tile_pool(name="ps", bufs=1, space="PSUM") as ps:
            kT = qkT[4]

            def rope_slice(n):
                sl = slice(n * 512, (n + 1) * 512)
                for m in (4, 0, 1, 2, 3):
                    swp = ps.tile([P, 512], F32, name="swp", tag="swp", bufs=1)
                    nc.tensor.matmul(swp, lhsT=perm, rhs=qkT[m][:, sl],
                                     start=True, stop=True)
                    t1 = rpool.tile([P, 512], F32, name="t1", tag="t1")
                    nc.vector.tensor_mul(t1, qkT[m][:, sl].bitcast(F32), ropeC[:, sl])
                    t2 = rpool.tile([P, 512], F32, name="t2", tag="t2")
                    nc.vector.tensor_mul(t2, swp, ropeS[:, sl])
                    nc.vector.tensor_add(qkT[m][:, sl], t1, t2)

            rope_slice(0)
            for n in range(NSL):
                sl = slice(n * 512, (n + 1) * 512)
                # attention for all 4 heads on this slice
                ntk = 4 * n + 4
                for h in range(4):
                    qTh = qkT[h]
                    yps = ps.tile([P, 512], F32, name="yps", tag="ypo", bufs=2)
                    lps = ps.tile([1, 512], F32, name="lps", tag="lps", bufs=1)
                    for tk in range(ntk):
                        j = tk - 4 * n
                        c0 = j * P if j >= 1 else 0    # first valid local column
                        stp = ps.tile([P, 512], F32, name="stp", tag="stp", bufs=4)
                        nc.tensor.matmul(
                            stp[:, c0:], lhsT=kT[:, tk * P:(tk + 1) * P],
                            rhs=qTh[:, n * 512 + c0:(n + 1) * 512],
                            start=True, stop=True)
                        if j >= 0:
                            nc.vector.tensor_add(
                                stp[:, c0:c0 + P], stp[:, c0:c0 + P], tri)
                        pt = ptpool.tile([P, 512], FR, name="pt", tag="pt")
                        nc.scalar.activation(out=pt[:, c0:], in_=stp[:, c0:],
                                             func=Exp, scale=SCALE)
                        st, sp = (tk == 0), (tk == ntk - 1)
                        nc.tensor.matmul(yps[:, c0:], lhsT=v_sb[:, tk, :],
                                         rhs=pt[:, c0:], start=st, stop=sp)
                        nc.tensor.matmul(lps[:, c0:], lhsT=onesc,
                                         rhs=pt[:, c0:], start=st, stop=sp)
                    linv = lpool.tile([1, 512], F32, name="linv", tag="linv")
                    nc.vector.reciprocal(out=linv, in_=lps)
                    bcsb = ytpool.tile([P, 512], F32, name="bcsb", tag="bcsb")
                    nc.gpsimd.partition_broadcast(bcsb, linv)
                    ysb = ytpool.tile([P, 512], F32, name="ysb", tag="ysb")
                    nc.any.tensor_copy(out=ysb, in_=yps)
                    nc.vector.tensor_mul(yT[h][:, sl], ysb, bcsb)
                # output projection for this slice's 4 row-tiles
                for t in range(4 * n, 4 * n + 4):
                    outsb = opool.tile([P, D], F32, name="outsb", tag="outsb")
                    for dsl in range(4):
                        ops = ps.tile([P, 512], F32, name="ops", tag="ypo", bufs=2)
                        for c in range(4):
                            nc.tensor.matmul(
                                ops, lhsT=yT[c][:, t * P:(t + 1) * P],
                                rhs=wo_sb[:, c, dsl * 512:(dsl + 1) * 512],
                                start=(c == 0), stop=(c == 3))
                        cp = nc.vector.tensor_copy if n >= 2 else nc.any.tensor_copy
                        cp(out=outsb[:, dsl * 512:(dsl + 1) * 512], in_=ops)
                        nc.sync.dma_start(
                            out=outp_d[t * P:(t + 1) * P, dsl * 512:(dsl + 1) * 512],
                            in_=outsb[:, dsl * 512:(dsl + 1) * 512])
                # next slice's rope after this slice's output projection
                if t == 4 * n + 3 and dsl == 3 and n + 1 < NSL:
                    rope_slice(n + 1)

    nc.compile()
    return nc


def _host_consts(freqs_cos, freqs_sin):
    C = np.repeat(np.asarray(freqs_cos, np.float32).T, 2, axis=0)
    S = np.repeat(np.asarray(freqs_sin, np.float32).T, 2, axis=0).copy()
    S[0::2] *= -1.0
    C = np.ascontiguousarray(C)
    S = np.ascontiguousarray(S)
    perm = np.zeros((P, P), np.float32)
    perm[np.arange(P), np.arange(P) ^ 1] = 1.0
    tri = np.where(np.arange(P)[:, None] <= np.arange(P)[None, :], 0.0, NEG).astype(np.float32)
    onesc = np.ones((P, 1), np.float32)
    onesr = np.ones((1, P), np.float32)
    return C, S, perm, tri, onesc, onesr


def _in_maps(x, freqs_cos, freqs_sin, Wq, Wk, Wv, Wo):
    C, S, perm, tri, onesc, onesr = _host_consts(freqs_cos, freqs_sin)
    xTb = [np.ascontiguousarray(np.asarray(x, np.float32)[b].T) for b in range(B)]
    Wq = np.asarray(Wq, np.float32); Wk = np.asarray(Wk, np.float32)
    Wv = np.asarray(Wv, np.float32); Wo = np.asarray(Wo, np.float32)
    ident = np.eye(P, dtype=np.float32)
    maps = []
    for b in range(B):
        for g in range(KVH):
            wqkv = np.ascontiguousarray(np.concatenate(
                [Wq[:, 512 * g:512 * (g + 1)], Wk[:, HD * g:HD * (g + 1)],
                 Wv[:, HD * g:HD * (g + 1)]], axis=1))
            maps.append({
                "xT": xTb[b],
                "wqkv": wqkv,
                "wo": np.ascontiguousarray(Wo[512 * g:512 * (g + 1), :]),
                "ropeC": C, "ropeS": S, "perm": perm, "tri": tri,
                "onesc": onesc, "onesr": onesr, "ident": ident,
            })
    return maps


def kernel(x, freqs_cos, freqs_sin, Wq, Wk, Wv, Wo):
    from concourse.bass_utils import run_bass_kernel_spmd

    if "nc" not in _CACHE:
        _CACHE["nc"] = _build_nc()
    nc = _CACHE["nc"]

    in_maps = _in_maps(x, freqs_cos, freqs_sin, Wq, Wk, Wv, Wo)
    res = None
    for attempt in range(3):
        try:
            res = run_bass_kernel_spmd(nc, in_maps, core_ids=list(range(8)))
            break
        except Exception:
            if attempt == 2:
                raise
    assert res is not None
    out = np.zeros((B, T, D), np.float32)
    for b in range(B):
        for g in range(KVH):
            out[b] += res.results[b * KVH + g]["outp"]
    return out


# revision 84
# speedup vs baseline: 1.0044x; 1.0032x over previous
"""Causal self-attention (GQA + RoPE) Trainium2 Bass kernel.

Problem: B=2, T=2048, D=2048, H=16 q-heads, KVH=4 kv-heads, HD=128.
Sharding: 8 cores = batch(2) x kv-groups(4). Core (b, g) computes q-heads
4g..4g+3 and kv-head g for batch b, producing a partial output
y_g @ Wo[512g:512g+512, :]; the host sums the 4 partials per batch.

Device-side layout (per core):
  xT   = x[b].T                       [D, T]     (host-transposed)
  qkT  = wqk.T @ xT                   5x[128, T] (q/k transposed: head-dim on partitions)
  v    = x[b] @ Wv_g                  [T, 128]   (natural)
  RoPE via pair-swap matmul (PERM) + elementwise maps C/S (host-precomputed)
  S^T tiles = kT_tile.T @ qT          [tk=128, tq<=512]  -> exp (ACT, scaled) -> P^T
  y^T[:, sl] += v_tile.T @ P^T ;  l[1, sl] += ones.T @ P^T   (PSUM accumulate)
  y^T *= broadcast(1/l)  (GPSIMD partition_broadcast of DVE reciprocal + DVE mul)
  outp = sum_c yT_c.T @ Wo_g[c-block]            [T, D]  partial

Slice-pipelined: for each 512-query slice, 4 heads' attention -> this
slice's output-projection rows -> next slice's rope, so PE/ACT/DVE/DMA
overlap across stages (rope DVE work drains while PE projects; v is
projected first so its PE transposes unblock early; output rows DMA out
per 512-column chunk as each PSUM copy lands). The first two projection
slices run kb-outer so matmuls consume each weight/x tile as its DMA
lands. PSUM budget
(8 banks): qk-proj 6 + v-transpose 2 during projection; then stp 4 +
yps/ops shared 2 + swp 1 + lps 1 (yps and ops have disjoint windows
within a slice, so sharing their two banks double-buffers both).

All matmuls run as float32r (FP22 single-pass); matmul-feeding tiles are
declared float32r and every producer writes the float32r view (walrus
birverifier requires rounded producers). Causality is handled by narrowing
the diagonal tiles' matmuls to the valid column range plus a triangular
mask add on the 128-wide diagonal block.
"""

import sys
import numpy as np

if "/opt/trn_rl_repo" not in sys.path:
    sys.path.insert(0, "/opt/trn_rl_repo")

B, T, D = 2, 2048, 2048
H, KVH = 16, 4
HD = 128
P = 128
NKB = D // P            # 16 contraction blocks
NTK = T // P            # 16 key tiles
NSL = T // 512          # 4 query slices of 512
SCALE = float(1.0 / np.sqrt(HD))
NEG = -1.0e30

_CACHE = {}


def _build_nc():
    import concourse.mybir as mybir
    import concourse.tile as tile
    from concourse import bacc
    from contextlib import ExitStack

    F32 = mybir.dt.float32
    FR = mybir.dt.float32r
    Exp = mybir.ActivationFunctionType.Exp

    nc = bacc.Bacc("TRN2", target_bir_lowering=False, debug=False)

    xT_d = nc.dram_tensor("xT", [D, T], F32, kind="ExternalInput").ap()
    wqkv_d = nc.dram_tensor("wqkv", [D, 768], F32, kind="ExternalInput").ap()
    wo_d = nc.dram_tensor("wo", [512, D], F32, kind="ExternalInput").ap()
    ropeC_d = nc.dram_tensor("ropeC", [P, T], F32, kind="ExternalInput").ap()
    ropeS_d = nc.dram_tensor("ropeS", [P, T], F32, kind="ExternalInput").ap()
    perm_d = nc.dram_tensor("perm", [P, P], F32, kind="ExternalInput").ap()
    tri_d = nc.dram_tensor("tri", [P, P], F32, kind="ExternalInput").ap()
    onesc_d = nc.dram_tensor("onesc", [P, 1], F32, kind="ExternalInput").ap()
    onesr_d = nc.dram_tensor("onesr", [1, P], F32, kind="ExternalInput").ap()
    ident_d = nc.dram_tensor("ident", [P, P], F32, kind="ExternalInput").ap()
    outp_d = nc.dram_tensor("outp", [T, D], F32, kind="ExternalOutput").ap()

    with tile.TileContext(nc) as tc, ExitStack() as ctx:
        # ---- persistent pools -------------------------------------------
        singles = ctx.enter_context(tc.tile_pool(name="singles", bufs=1))
        qk_pool = ctx.enter_context(tc.tile_pool(name="qk", bufs=1))
        v_pool = ctx.enter_context(tc.tile_pool(name="vp", bufs=1))

        ropeC = singles.tile([P, T], F32)
        ropeS = singles.tile([P, T], F32)
        perm = singles.tile([P, P], FR)
        nc.gpsimd.dma_start(out=perm, in_=perm_d.bitcast(FR))
        tri = singles.tile([P, P], F32)
        nc.gpsimd.dma_start(out=tri, in_=tri_d)
        onesc = singles.tile([P, 1], FR)
        nc.gpsimd.dma_start(out=onesc, in_=onesc_d.bitcast(FR))
        onesr = singles.tile([1, P], FR)
        nc.gpsimd.dma_start(out=onesr, in_=onesr_d.bitcast(FR))
        ident = singles.tile([P, P], F32)
        nc.gpsimd.dma_start(out=ident, in_=ident_d)

        # qkT[m]: m=0..3 q-heads, m=4 k-head; vT = v transposed [HD, T];
        # v_sb[:, tk, :] = v[128tk:128tk+128, :] natural (for PV lhsT)
        qkT = [qk_pool.tile([P, T], FR, name=f"qkT{m}") for m in range(5)]
        vT = qk_pool.tile([P, T], F32, name="vT")
        v_sb = v_pool.tile([P, NTK, HD], FR)

        # ---- phase 1: projections + v transpose --------------------------
        with tc.tile_pool(name="wqkv", bufs=1) as wpool, \
             tc.tile_pool(name="xts", bufs=32) as xpool, \
             tc.tile_pool(name="qkps", bufs=1, space="PSUM") as qkps_pool, \
             tc.tile_pool(name="vtps", bufs=2, space="PSUM") as vtpool:
            wqkv_sb = wpool.tile([P, NKB, 768], FR)
            wqkv_r = wqkv_d.rearrange("(kb p) m -> p kb m", p=P).bitcast(FR)
            for kb in range(NKB):
                nc.scalar.dma_start(out=wqkv_sb[:, kb, :], in_=wqkv_r[:, kb, :])

            for n in range(NSL):
                xts = []
                for kb in range(NKB):
                    xt = xpool.tile([P, 512], FR, name="xt", tag="xt")
                    nc.sync.dma_start(
                        out=xt,
                        in_=xT_d[kb * P:(kb + 1) * P, n * 512:(n + 1) * 512].bitcast(FR))
                    xts.append(xt)
                if n <= 1:
                    # kb-outer on the first two slices: consume each weight and
                    # xt tile as its DMA lands instead of waiting for the batch
                    qkps6 = [qkps_pool.tile([P, 512], F32, name=f"qkps{m}",
                                            tag=f"qkps{m}") for m in range(6)]
                    for kb in range(NKB):
                        for m in range(6):
                            nc.tensor.matmul(
                                qkps6[m], lhsT=wqkv_sb[:, kb, m * P:(m + 1) * P],
                                rhs=xts[kb], start=(kb == 0), stop=(kb == NKB - 1))
                    nc.any.tensor_copy(
                        out=vT[:, n * 512:(n + 1) * 512], in_=qkps6[5])
                    for m in range(5):
                        nc.any.tensor_copy(
                            out=qkT[m][:, n * 512:(n + 1) * 512], in_=qkps6[m])
                else:
                    # m-outer: each m's psum group closes early so its copy-out
                    # overlaps the next m's matmuls
                    for m in (5, 0, 1, 2, 3, 4):
                        qkps = qkps_pool.tile([P, 512], F32, name=f"qkps{m}",
                                              tag=f"qkps{m}")
                        for kb in range(NKB):
                            nc.tensor.matmul(
                                qkps, lhsT=wqkv_sb[:, kb, m * P:(m + 1) * P],
                                rhs=xts[kb], start=(kb == 0), stop=(kb == NKB - 1))
                        if m < 5:
                            nc.any.tensor_copy(
                                out=qkT[m][:, n * 512:(n + 1) * 512], in_=qkps)
                        else:
                            nc.any.tensor_copy(
                                out=vT[:, n * 512:(n + 1) * 512], in_=qkps)
                for tk in range(4 * n, 4 * n + 4):
                    vtp = vtpool.tile([P, P], F32, name="vtp", tag="vtp")
                    nc.tensor.transpose(vtp, in_=vT[:, tk * P:(tk + 1) * P],
                                        identity=ident)
                    nc.any.tensor_copy(out=v_sb[:, tk, :], in_=vtp)
                if n == 0:
                    # rope maps: needed from the rope of slice 0 (~25us in);
                    # emitted after slice-0 proj so they don't delay startup
                    nc.scalar.dma_start(out=ropeC, in_=ropeC_d)
                    nc.scalar.dma_start(out=ropeS, in_=ropeS_d)

        # wo load here: overlaps rope + early attention, after proj DMAs
        y_pool = ctx.enter_context(tc.tile_pool(name="yp", bufs=1))
        yT = [y_pool.tile([P, T], FR, name=f"yT{h}") for h in range(4)]
        wos_pool = ctx.enter_context(tc.tile_pool(name="wos", bufs=1))
        wo_sb = wos_pool.tile([P, 4, D], FR)
        nc.scalar.dma_start(
            out=wo_sb, in_=wo_d.rearrange("(c p) d -> p c d", p=P).bitcast(FR))

        # ---- phase 2+3: rope, attention, output proj, slice-pipelined ----
        with tc.tile_pool(name="ropet", bufs=3) as rpool, \
             tc.tile_pool(name="pts", bufs=4) as ptpool, \
             tc.tile_pool(name="lsb", bufs=2) as lpool, \
             tc.tile_pool(name="ytmp", bufs=3) as ytpool, \
             tc.tile_pool(name="osb", bufs=3) as opool, \
             tc.tile_pool(name="ps", bufs=1, space="PSUM") as ps:
            kT = qkT[4]

            def rope_slice(n):
                sl = slice(n * 512, (n + 1) * 512)
                for m in (4, 0, 1, 2, 3):
                    swp = ps.tile([P, 512], F32, name="swp", tag="swp", bufs=1)
                    nc.tensor.matmul(swp, lhsT=perm, rhs=qkT[m][:, sl],
                                     start=True, stop=True)
                    t1 = rpool.tile([P, 512], F32, name="t1", tag="t1")
                    nc.vector.tensor_mul(t1, qkT[m][:, sl].bitcast(F32), ropeC[:, sl])
                    t2 = rpool.tile([P, 512], F32, name="t2", tag="t2")
                    nc.vector.tensor_mul(t2, swp, ropeS[:, sl])
                    nc.vector.tensor_add(qkT[m][:, sl], t1, t2)

            rope_slice(0)
            for n in range(NSL):
                sl = slice(n * 512, (n + 1) * 512)
                # attention for all 4 heads on this slice
                ntk = 4 * n + 4
                for h in range(4):
                    qTh = qkT[h]
                    yps = ps.tile([P, 512], F32, name="yps", tag="ypo", bufs=2)
                    lps = ps.tile([1, 512], F32, name="lps", tag="lps", bufs=1)
                    for tk in range(ntk):
                        j = tk - 4 * n
                        c0 = j * P if j >= 1 else 0    # first valid local column
                        stp = ps.tile([P, 512], F32, name="stp", tag="stp", bufs=4)
                        nc.tensor.matmul(
                            stp[:, c0:], lhsT=kT[:, tk * P:(tk + 1) * P],
                            rhs=qTh[:, n * 512 + c0:(n + 1) * 512],
                            start=True, stop=True)
                        if j >= 0:
                            nc.vector.tensor_add(
                                stp[:, c0:c0 + P], stp[:, c0:c0 + P], tri)
                        pt = ptpool.tile([P, 512], FR, name="pt", tag="pt")
                        nc.scalar.activation(out=pt[:, c0:], in_=stp[:, c0:],
                                             func=Exp, scale=SCALE)
                        st, sp = (tk == 0), (tk == ntk - 1)
                        nc.tensor.matmul(yps[:, c0:], lhsT=v_sb[:, tk, :],
                                         rhs=pt[:, c0:], start=st, stop=sp)
                        nc.tensor.matmul(lps[:, c0:], lhsT=onesc,
                                         rhs=pt[:, c0:], start=st, stop=sp)
                    linv = lpool.tile([1, 512], F32, name="linv", tag="linv")
                    nc.vector.reciprocal(out=linv, in_=lps)
                    bcsb = ytpool.tile([P, 512], F32, name="bcsb", tag="bcsb")
                    nc.gpsimd.partition_broadcast(bcsb, linv)
                    ysb = ytpool.tile([P, 512], F32, name="ysb", tag="ysb")
                    nc.scalar.copy(out=ysb, in_=yps)
                    nc.vector.tensor_mul(yT[h][:, sl], ysb, bcsb)
                # output projection for this slice's 4 row-tiles
                for t in range(4 * n, 4 * n + 4):
                    outsb = opool.tile([P, D], F32, name="outsb", tag="outsb")
                    for dsl in range(4):
                        ops = ps.tile([P, 512], F32, name="ops", tag="ypo", bufs=2)
                        for c in range(4):
                            nc.tensor.matmul(
                                ops, lhsT=yT[c][:, t * P:(t + 1) * P],
                                rhs=wo_sb[:, c, dsl * 512:(dsl + 1) * 512],
                                start=(c == 0), stop=(c == 3))
                        cp = nc.vector.tensor_copy if n >= 2 else nc.any.tensor_copy
                        cp(out=outsb[:, dsl * 512:(dsl + 1) * 512], in_=ops)
                        nc.sync.dma_start(
                            out=outp_d[t * P:(t + 1) * P, dsl * 512:(dsl + 1) * 512],
                            in_=outsb[:, dsl * 512:(dsl + 1) * 512])
                # next slice's rope after this slice's output projection
                if t == 4 * n + 3 and dsl == 3 and n + 1 < NSL:
                    rope_slice(n + 1)

    nc.compile()
    return nc


def _host_consts(freqs_cos, freqs_sin):
    C = np.repeat(np.asarray(freqs_cos, np.float32).T, 2, axis=0)
    S = np.repeat(np.asarray(freqs_sin, np.float32).T, 2, axis=0).copy()
    S[0::2] *= -1.0
    C = np.ascontiguousarray(C)
    S = np.ascontiguousarray(S)
    perm = np.zeros((P, P), np.float32)
    perm[np.arange(P), np.arange(P) ^ 1] = 1.0
    tri = np.where(np.arange(P)[:, None] <= np.arange(P)[None, :], 0.0, NEG).astype(np.float32)
    onesc = np.ones((P, 1), np.float32)
    onesr = np.ones((1, P), np.float32)
    return C, S, perm, tri, onesc, onesr


def _in_maps(x, freqs_cos, freqs_sin, Wq, Wk, Wv, Wo):
    C, S, perm, tri, onesc, onesr = _host_consts(freqs_cos, freqs_sin)
    xTb = [np.ascontiguousarray(np.asarray(x, np.float32)[b].T) for b in range(B)]
    Wq = np.asarray(Wq, np.float32); Wk = np.asarray(Wk, np.float32)
    Wv = np.asarray(Wv, np.float32); Wo = np.asarray(Wo, np.float32)
    ident = np.eye(P, dtype=np.float32)
    maps = []
    for b in range(B):
        for g in range(KVH):
            wqkv = np.ascontiguousarray(np.concatenate(
                [Wq[:, 512 * g:512 * (g + 1)], Wk[:, HD * g:HD * (g + 1)],
                 Wv[:, HD * g:HD * (g + 1)]], axis=1))
            maps.append({
                "xT": xTb[b],
                "wqkv": wqkv,
                "wo": np.ascontiguousarray(Wo[512 * g:512 * (g + 1), :]),
                "ropeC": C, "ropeS": S, "perm": perm, "tri": tri,
                "onesc": onesc, "onesr": onesr, "ident": ident,
            })
    return maps


def kernel(x, freqs_cos, freqs_sin, Wq, Wk, Wv, Wo):
    from concourse.bass_utils import run_bass_kernel_spmd

    if "nc" not in _CACHE:
        _CACHE["nc"] = _build_nc()
    nc = _CACHE["nc"]

    in_maps = _in_maps(x, freqs_cos, freqs_sin, Wq, Wk, Wv, Wo)
    res = None
    for attempt in range(3):
        try:
            res = run_bass_kernel_spmd(nc, in_maps, core_ids=list(range(8)))
            break
        except Exception:
            if attempt == 2:
                raise
    assert res is not None
    out = np.zeros((B, T, D), np.float32)
    for b in range(B):
        for g in range(KVH):
            out[b] += res.results[b * KVH + g]["outp"]
    return out
